# revision 5
# baseline (speedup 1.0000x reference)
import numpy as np
import ml_dtypes

# ---- problem constants (hardcoded from spec) ----
B, C, H, W = 2, 128, 256, 512
P = B * H * W
TEMPERATURE = 0.1
BASE_TEMPERATURE = 0.07
MAX_SAMPLES = 1024
MAX_VIEWS = 100
NUM_CLASSES = 8
N = NUM_CLASSES * MAX_SAMPLES
N_CORES = 8
BLK = N // N_CORES
SCALE = np.float32(BASE_TEMPERATURE / (TEMPERATURE * TEMPERATURE))  # 7.0f

NK = 5
KC = NK * BLK               # 5120 columns per core

A16_7 = np.float32(7.0 * 128.0 / np.log(2.0))
B16C = np.float32(127.0 * 128.0 - 7.25)
A8_7 = np.float32(7.0 * 8.0 / np.log(2.0))
B8C = np.float32(56.0 - 0.45)

MM_W = 512
V = 2432                    # DVE (schraudolph) exp columns per chunk
RS = 4096                   # psum ring (8 banks)
EA_FP8 = True               # ship ACT-A exp tiles as fp8_e4m3

_PROGRAM = {}
_STATE = {}


def _sample_indices_host(labels_flat_np):
    import jax
    import jax.numpy as jnp

    cpu = jax.devices("cpu")[0]
    with jax.default_device(cpu):
        labels_flat = jnp.asarray(labels_flat_np)
        key = jax.random.key(42)
        k1, k2 = jax.random.split(key)
        scores = jax.random.uniform(k1, (P,))
        class_mask = (
            labels_flat[None, :]
            == jnp.arange(NUM_CLASSES, dtype=labels_flat.dtype)[:, None]
        )
        masked_scores = jnp.where(class_mask, scores[None, :], -1.0)
        _, idx = jax.lax.top_k(masked_scores, MAX_SAMPLES)
        sampled_idx = idx.reshape(-1)
        row_scores = jax.random.uniform(k2, (N, MAX_SAMPLES))
        _, sel = jax.lax.top_k(row_scores, MAX_VIEWS)
        block_start = (jnp.arange(N) // MAX_SAMPLES) * MAX_SAMPLES
        pos_cols = sel + block_start[:, None]
        return np.asarray(sampled_idx), np.asarray(pos_cols)


def _ring_runs(sb, lo, hi):
    """Contiguous psum-ring runs for chunk cols [lo,hi);
    ring col of chunk col x = (sb*512 + x) mod RS."""
    base = sb * 512
    cuts = [w - base for w in (RS, 2 * RS) if lo < w - base < hi]
    edges = [lo] + cuts + [hi]
    return [(((base + l) % RS), l, h) for l, h in zip(edges[:-1], edges[1:])]


def _build_program():
    """v4: PE = main matmuls only, into an 8-bank psum ring.
    exp split: DVE schraudolph (cols [0,V), int16 bit trick) and ACT exact
    (cols [V,5120)). Columns [0,4096) of every chunk are shipped to DRAM
    (idle DMA engines) and reduced on the host (cs of k1..k3, row sums,
    exact diagonal removal). On-device: only k4 row sums via a 4x-mode
    tensor_scalar accumulate."""
    if _PROGRAM:
        return _PROGRAM

    import concourse.mybir as mybir
    from concourse import bacc, tile

    f32 = mybir.dt.float32
    bf16 = mybir.dt.bfloat16
    i16 = mybir.dt.int16
    AO = mybir.AluOpType

    nc = bacc.Bacc("TRN2", target_bir_lowering=False)

    embR_d = nc.dram_tensor("embR", [128, KC], bf16, kind="ExternalInput")
    fp8 = mybir.dt.float8e4
    i8 = mybir.dt.int8
    adt = fp8 if EA_FP8 else bf16
    e0_d = nc.dram_tensor("e0", [128, 8 * V], i8, kind="ExternalOutput")
    eA_d = nc.dram_tensor("eA", [128, 8 * (4096 - V)], adt, kind="ExternalOutput")
    eB_d = nc.dram_tensor("eB", [128, 8 * 1024], adt, kind="ExternalOutput")

    with tile.TileContext(nc) as tc:
        with (
            tc.tile_pool(name="persist", bufs=1) as pp,
            tc.tile_pool(name="psum", bufs=1, space="PSUM") as ps,
        ):
            embR = pp.tile([128, KC], bf16)
            ering0 = pp.tile([128, 3 * V], i8)              # [0,V) 3 buffers
            eringA = pp.tile([128, 3 * (4096 - V)], adt)    # [V,4096) 3 buffers
            eringB = pp.tile([128, 3 * 1024], adt)          # k4 3 buffers
            ring = ps.tile([128, RS], f32)

            warm = pp.tile([128, 512], bf16)
            for lo, hi in [(0, 1024), (1024, 2048), (2048, 3072),
                           (3072, 4096), (4096, KC)]:
                nc.sync.dma_start(out=embR[:, lo:hi], in_=embR_d[:, lo:hi])
            # PE warm-up: ramp the p-state while input DMAs land; outputs
            # are overwritten by the real matmuls.
            nc.vector.memset(warm[:], 1.0)
            for _ in range(6):
                nc.tensor.matmul(
                    ring[:, 0:512], warm[:, 0:128], warm[:, 0:512],
                    start=True, stop=True,
                )

            for c in range(8):
                sb = (10 * c) % 8
                h0 = (c % 3) * V
                hA = (c % 3) * (4096 - V)
                lhsT = embR[:, c * 128:(c + 1) * 128]

                def mm(x0, x1):
                    for x in range(x0, x1, MM_W):
                        rc = (sb * 512 + x) % RS
                        nc.tensor.matmul(
                            ring[:, rc:rc + MM_W], lhsT, embR[:, x:x + MM_W],
                            start=True, stop=True,
                        )

                # split DVE exp: [0,1024) first so the k4 matmuls (which
                # reuse those two ring slots) only wait on this short read
                mm(0, 1024)
                for rc, l, h in _ring_runs(sb, 0, 1024):
                    nc.vector.tensor_scalar(
                        out=ering0[:, h0 + l:h0 + h],
                        in0=ring[:, rc:rc + (h - l)],
                        scalar1=float(A8_7), scalar2=float(B8C),
                        op0=AO.mult, op1=AO.add,
                    )
                mm(1024, ((V + 511) // 512) * 512)
                for rc, l, h in _ring_runs(sb, 1024, V):
                    nc.vector.tensor_scalar(
                        out=ering0[:, h0 + l:h0 + h],
                        in0=ring[:, rc:rc + (h - l)],
                        scalar1=float(A8_7), scalar2=float(B8C),
                        op0=AO.mult, op1=AO.add,
                    )
                nc.sync.dma_start(
                    out=e0_d[:, c * V:(c + 1) * V],
                    in_=ering0[:, h0:h0 + V],
                )
                mm(((V + 511) // 512) * 512, RS)
                for rc, l, h in _ring_runs(sb, V, 4096):
                    nc.scalar.activation(
                        eringA[:, hA + l - V:hA + h - V],
                        ring[:, rc:rc + (h - l)],
                        mybir.ActivationFunctionType.Exp, scale=float(SCALE),
                    )
                nc.sync.dma_start(
                    out=eA_d[:, c * (4096 - V):(c + 1) * (4096 - V)],
                    in_=eringA[:, hA:hA + 4096 - V],
                )
                mm(RS, KC)   # k4; reuses ring slots [0,1024) after DVE read
                hB = (c % 3) * 1024
                for rc, l, h in _ring_runs(sb, 4096, KC):
                    nc.scalar.activation(
                        eringB[:, hB + l - 4096:hB + h - 4096],
                        ring[:, rc:rc + (h - l)],
                        mybir.ActivationFunctionType.Exp, scale=float(SCALE),
                    )
                nc.sync.dma_start(
                    out=eB_d[:, c * 1024:(c + 1) * 1024],
                    in_=eringB[:, hB:hB + 1024],
                )


    nc.finalize()
    _PROGRAM["nc"] = nc
    return _PROGRAM


def _spos_host(emb_n, pos_cols):
    rows = np.repeat(np.arange(N), MAX_VIEWS)
    cols = pos_cols.ravel()
    mask = cols != rows
    rows, cols = rows[mask], cols[mask]
    total = 0.0
    for ofs in range(0, rows.size, 131072):
        r = rows[ofs:ofs + 131072]
        c = cols[ofs:ofs + 131072]
        dots = np.einsum("ij,ij->i", emb_n[r], emb_n[c], dtype=np.float64)
        total += float(np.exp(np.float64(SCALE) * dots).sum())
    return total


def _host_prep(embeddings, labels):
    sampled_idx, pos_cols = _sample_indices_host(labels.reshape(-1))
    hw = H * W
    b = sampled_idx // hw
    h = (sampled_idx % hw) // W
    w = sampled_idx % W
    emb_s = embeddings[b, :, h, w].astype(np.float32)
    norm = np.sqrt(np.sum(emb_s * emb_s, axis=1, dtype=np.float32)).astype(np.float32)
    norm = np.maximum(norm, np.float32(1e-12))
    emb_n = emb_s / norm[:, None]
    embT_bf = np.ascontiguousarray(emb_n.T).astype(ml_dtypes.bfloat16)

    spos = _spos_host(emb_n, pos_cols)

    in_maps = []
    for m in range(N_CORES):
        embR = np.ascontiguousarray(np.roll(embT_bf, -BLK * m, axis=1)[:, :KC])
        in_maps.append({"embR": embR})
    return in_maps, spos


def _combine(results, spos_total):
    rt = []
    csv = []
    for res in results:
        codes = np.asarray(res["e0"]).view(np.int8)
        codes = np.where((codes < 0) | (codes == 127), np.int8(0), codes)
        e0 = codes.view(ml_dtypes.float8_e4m3).astype(np.float32).reshape(128, 8, V)
        eA = np.asarray(res["eA"], dtype=np.float32).reshape(128, 8, 4096 - V)
        eB = np.asarray(res["eB"], dtype=np.float32).reshape(128, 8, 1024)

        rs = (e0.sum(axis=2, dtype=np.float64)
              + eA.sum(axis=2, dtype=np.float64)
              + eB.sum(axis=2, dtype=np.float64))                # [128, 8]
        # subtract the diagonal exactly as shipped (cancels even for
        # saturated codes)
        p_ = np.arange(128)
        for c in range(8):
            rs[p_, c] -= e0[p_, c, c * 128 + p_].astype(np.float64)
        rt.append(rs.T.reshape(-1))                              # u = c*128+p
        cs = np.concatenate([
            e0[:, :, 1024:].sum(axis=(0, 1), dtype=np.float64),
            eA.sum(axis=(0, 1), dtype=np.float64),
        ])
        csv.append(cs.reshape(3, 1024))
    col_sum = np.empty(N, dtype=np.float64)
    for b_ in range(N_CORES):
        s = rt[b_].copy()
        for k in (1, 2, 3):
            s += csv[(b_ - k) % N_CORES][k - 1]
        col_sum[b_ * BLK:(b_ + 1) * BLK] = s
    loss = -np.log(spos_total) + np.mean(np.log(col_sum))
    return np.float32(loss)


def kernel(embeddings: np.ndarray, labels: np.ndarray) -> np.ndarray:
    from concourse.bass_utils import run_bass_kernel_spmd

    prog = _build_program()
    in_maps, spos = _host_prep(np.asarray(embeddings), np.asarray(labels))
    out = run_bass_kernel_spmd(prog["nc"], in_maps, list(range(N_CORES)))
    return _combine(out.results, spos)


# revision 6
# speedup vs baseline: 1.0572x; 1.0572x over previous
import numpy as np
import ml_dtypes

# ---- problem constants (hardcoded from spec) ----
B, C, H, W = 2, 128, 256, 512
P = B * H * W
TEMPERATURE = 0.1
BASE_TEMPERATURE = 0.07
MAX_SAMPLES = 1024
MAX_VIEWS = 100
NUM_CLASSES = 8
N = NUM_CLASSES * MAX_SAMPLES
N_CORES = 8
BLK = N // N_CORES
SCALE = np.float32(BASE_TEMPERATURE / (TEMPERATURE * TEMPERATURE))  # 7.0f

NK = 5
KC = NK * BLK               # 5120 columns per core

A16_7 = np.float32(7.0 * 128.0 / np.log(2.0))
B16C = np.float32(127.0 * 128.0 - 7.25)
A8_7 = np.float32(7.0 * 8.0 / np.log(2.0))
B8C = np.float32(56.0 - 0.45)

MM_W = 512
V = 2432                    # DVE (schraudolph) exp columns per chunk
RS = 4096                   # psum ring (8 banks)
EA_FP8 = True               # ship ACT-A exp tiles as fp8_e4m3

_PROGRAM = {}
_STATE = {}


def _sample_indices_host(labels_flat_np):
    import jax
    import jax.numpy as jnp

    cpu = jax.devices("cpu")[0]
    with jax.default_device(cpu):
        labels_flat = jnp.asarray(labels_flat_np)
        key = jax.random.key(42)
        k1, k2 = jax.random.split(key)
        scores = jax.random.uniform(k1, (P,))
        class_mask = (
            labels_flat[None, :]
            == jnp.arange(NUM_CLASSES, dtype=labels_flat.dtype)[:, None]
        )
        masked_scores = jnp.where(class_mask, scores[None, :], -1.0)
        _, idx = jax.lax.top_k(masked_scores, MAX_SAMPLES)
        sampled_idx = idx.reshape(-1)
        row_scores = jax.random.uniform(k2, (N, MAX_SAMPLES))
        _, sel = jax.lax.top_k(row_scores, MAX_VIEWS)
        block_start = (jnp.arange(N) // MAX_SAMPLES) * MAX_SAMPLES
        pos_cols = sel + block_start[:, None]
        return np.asarray(sampled_idx), np.asarray(pos_cols)


def _ring_runs(sb, lo, hi):
    """Contiguous psum-ring runs for chunk cols [lo,hi);
    ring col of chunk col x = (sb*512 + x) mod RS."""
    base = sb * 512
    cuts = [w - base for w in (RS, 2 * RS) if lo < w - base < hi]
    edges = [lo] + cuts + [hi]
    return [(((base + l) % RS), l, h) for l, h in zip(edges[:-1], edges[1:])]


def _build_program():
    """v4: PE = main matmuls only, into an 8-bank psum ring.
    exp split: DVE schraudolph (cols [0,V), int16 bit trick) and ACT exact
    (cols [V,5120)). Columns [0,4096) of every chunk are shipped to DRAM
    (idle DMA engines) and reduced on the host (cs of k1..k3, row sums,
    exact diagonal removal). On-device: only k4 row sums via a 4x-mode
    tensor_scalar accumulate."""
    if _PROGRAM:
        return _PROGRAM

    import concourse.mybir as mybir
    from concourse import bacc, tile

    f32 = mybir.dt.float32
    bf16 = mybir.dt.bfloat16
    i16 = mybir.dt.int16
    AO = mybir.AluOpType

    nc = bacc.Bacc("TRN2", target_bir_lowering=False)

    embR_d = nc.dram_tensor("embR", [128, KC], bf16, kind="ExternalInput")
    fp8 = mybir.dt.float8e4
    i8 = mybir.dt.int8
    adt = fp8 if EA_FP8 else bf16
    e0_d = nc.dram_tensor("e0", [128, 8 * V], i8, kind="ExternalOutput")
    eA_d = nc.dram_tensor("eA", [128, 8 * (4096 - V)], adt, kind="ExternalOutput")
    eB_d = nc.dram_tensor("eB", [128, 8 * 1024], adt, kind="ExternalOutput")

    with tile.TileContext(nc) as tc:
        with (
            tc.tile_pool(name="persist", bufs=1) as pp,
            tc.tile_pool(name="psum", bufs=1, space="PSUM") as ps,
        ):
            embR = pp.tile([128, KC], bf16)
            ering0 = pp.tile([128, 2 * V], i8)              # [0,V) 2 buffers
            eringA = pp.tile([128, 2 * (4096 - V)], adt)    # [V,4096) 2 buffers
            eringB = pp.tile([128, 2 * 1024], adt)          # k4 2 buffers
            ring = ps.tile([128, RS], f32)

            warm = pp.tile([128, 512], bf16)
            for lo, hi in [(0, 1024), (1024, 2048), (2048, 3072),
                           (3072, 4096), (4096, KC)]:
                nc.sync.dma_start(out=embR[:, lo:hi], in_=embR_d[:, lo:hi])
            # PE warm-up: ramp the p-state while input DMAs land; outputs
            # are overwritten by the real matmuls.
            nc.vector.memset(warm[:], 1.0)
            for _ in range(6):
                nc.tensor.matmul(
                    ring[:, 0:512], warm[:, 0:128], warm[:, 0:512],
                    start=True, stop=True,
                )

            for c in range(8):
                sb = (10 * c) % 8
                h0 = (c % 2) * V
                hA = (c % 2) * (4096 - V)
                lhsT = embR[:, c * 128:(c + 1) * 128]

                def mm(x0, x1):
                    for x in range(x0, x1, MM_W):
                        rc = (sb * 512 + x) % RS
                        nc.tensor.matmul(
                            ring[:, rc:rc + MM_W], lhsT, embR[:, x:x + MM_W],
                            start=True, stop=True,
                        )

                # split DVE exp: [0,1024) first so the k4 matmuls (which
                # reuse those two ring slots) only wait on this short read
                mm(0, 1024)
                for rc, l, h in _ring_runs(sb, 0, 1024):
                    nc.vector.tensor_scalar(
                        out=ering0[:, h0 + l:h0 + h],
                        in0=ring[:, rc:rc + (h - l)],
                        scalar1=float(A8_7), scalar2=float(B8C),
                        op0=AO.mult, op1=AO.add,
                    )
                nc.sync.dma_start(
                    out=e0_d[:, c * V:c * V + 1024],
                    in_=ering0[:, h0:h0 + 1024],
                )
                mm(1024, ((V + 511) // 512) * 512)
                for rc, l, h in _ring_runs(sb, 1024, V):
                    nc.vector.tensor_scalar(
                        out=ering0[:, h0 + l:h0 + h],
                        in0=ring[:, rc:rc + (h - l)],
                        scalar1=float(A8_7), scalar2=float(B8C),
                        op0=AO.mult, op1=AO.add,
                    )
                nc.sync.dma_start(
                    out=e0_d[:, c * V + 1024:(c + 1) * V],
                    in_=ering0[:, h0 + 1024:h0 + V],
                )
                mm(((V + 511) // 512) * 512, RS)
                for rc, l, h in _ring_runs(sb, V, 4096):
                    nc.scalar.activation(
                        eringA[:, hA + l - V:hA + h - V],
                        ring[:, rc:rc + (h - l)],
                        mybir.ActivationFunctionType.Exp, scale=float(SCALE),
                    )
                nc.sync.dma_start(
                    out=eA_d[:, c * (4096 - V):(c + 1) * (4096 - V)],
                    in_=eringA[:, hA:hA + 4096 - V],
                )
                mm(RS, KC)   # k4; reuses ring slots [0,1024) after DVE read
                hB = (c % 2) * 1024
                for rc, l, h in _ring_runs(sb, 4096, KC):
                    nc.scalar.activation(
                        eringB[:, hB + l - 4096:hB + h - 4096],
                        ring[:, rc:rc + (h - l)],
                        mybir.ActivationFunctionType.Exp, scale=float(SCALE),
                    )
                nc.sync.dma_start(
                    out=eB_d[:, c * 1024:(c + 1) * 1024],
                    in_=eringB[:, hB:hB + 1024],
                )


    nc.finalize()
    _PROGRAM["nc"] = nc
    return _PROGRAM


def _spos_host(emb_n, pos_cols):
    rows = np.repeat(np.arange(N), MAX_VIEWS)
    cols = pos_cols.ravel()
    mask = cols != rows
    rows, cols = rows[mask], cols[mask]
    total = 0.0
    for ofs in range(0, rows.size, 131072):
        r = rows[ofs:ofs + 131072]
        c = cols[ofs:ofs + 131072]
        dots = np.einsum("ij,ij->i", emb_n[r], emb_n[c], dtype=np.float64)
        total += float(np.exp(np.float64(SCALE) * dots).sum())
    return total


def _host_prep(embeddings, labels):
    sampled_idx, pos_cols = _sample_indices_host(labels.reshape(-1))
    hw = H * W
    b = sampled_idx // hw
    h = (sampled_idx % hw) // W
    w = sampled_idx % W
    emb_s = embeddings[b, :, h, w].astype(np.float32)
    norm = np.sqrt(np.sum(emb_s * emb_s, axis=1, dtype=np.float32)).astype(np.float32)
    norm = np.maximum(norm, np.float32(1e-12))
    emb_n = emb_s / norm[:, None]
    embT_bf = np.ascontiguousarray(emb_n.T).astype(ml_dtypes.bfloat16)

    spos = _spos_host(emb_n, pos_cols)

    in_maps = []
    for m in range(N_CORES):
        embR = np.ascontiguousarray(np.roll(embT_bf, -BLK * m, axis=1)[:, :KC])
        in_maps.append({"embR": embR})
    return in_maps, spos


def _combine(results, spos_total):
    rt = []
    csv = []
    for res in results:
        codes = np.asarray(res["e0"]).view(np.int8)
        codes = np.where((codes < 0) | (codes == 127), np.int8(0), codes)
        e0 = codes.view(ml_dtypes.float8_e4m3).astype(np.float32).reshape(128, 8, V)
        eA = np.asarray(res["eA"], dtype=np.float32).reshape(128, 8, 4096 - V)
        eB = np.asarray(res["eB"], dtype=np.float32).reshape(128, 8, 1024)

        rs = (e0.sum(axis=2, dtype=np.float64)
              + eA.sum(axis=2, dtype=np.float64)
              + eB.sum(axis=2, dtype=np.float64))                # [128, 8]
        # subtract the diagonal exactly as shipped (cancels even for
        # saturated codes)
        p_ = np.arange(128)
        for c in range(8):
            rs[p_, c] -= e0[p_, c, c * 128 + p_].astype(np.float64)
        rt.append(rs.T.reshape(-1))                              # u = c*128+p
        cs = np.concatenate([
            e0[:, :, 1024:].sum(axis=(0, 1), dtype=np.float64),
            eA.sum(axis=(0, 1), dtype=np.float64),
        ])
        csv.append(cs.reshape(3, 1024))
    col_sum = np.empty(N, dtype=np.float64)
    for b_ in range(N_CORES):
        s = rt[b_].copy()
        for k in (1, 2, 3):
            s += csv[(b_ - k) % N_CORES][k - 1]
        col_sum[b_ * BLK:(b_ + 1) * BLK] = s
    loss = -np.log(spos_total) + np.mean(np.log(col_sum))
    return np.float32(loss)


def kernel(embeddings: np.ndarray, labels: np.ndarray) -> np.ndarray:
    from concourse.bass_utils import run_bass_kernel_spmd

    prog = _build_program()
    in_maps, spos = _host_prep(np.asarray(embeddings), np.asarray(labels))
    out = run_bass_kernel_spmd(prog["nc"], in_maps, list(range(N_CORES)))
    return _combine(out.results, spos)


# revision 7
# speedup vs baseline: 1.0627x; 1.0052x over previous
import numpy as np
import ml_dtypes

# ---- problem constants (hardcoded from spec) ----
B, C, H, W = 2, 128, 256, 512
P = B * H * W
TEMPERATURE = 0.1
BASE_TEMPERATURE = 0.07
MAX_SAMPLES = 1024
MAX_VIEWS = 100
NUM_CLASSES = 8
N = NUM_CLASSES * MAX_SAMPLES
N_CORES = 8
BLK = N // N_CORES
SCALE = np.float32(BASE_TEMPERATURE / (TEMPERATURE * TEMPERATURE))  # 7.0f

NK = 5
KC = NK * BLK               # 5120 columns per core

A16_7 = np.float32(7.0 * 128.0 / np.log(2.0))
B16C = np.float32(127.0 * 128.0 - 7.25)
A8_7 = np.float32(7.0 * 8.0 / np.log(2.0))
B8C = np.float32(56.0 - 0.45)

MM_W = 512
V = 2432                    # DVE (schraudolph) exp columns per chunk
RS = 4096                   # psum ring (8 banks)
EA_FP8 = True               # ship ACT-A exp tiles as fp8_e4m3

_PROGRAM = {}
_STATE = {}


def _sample_indices_host(labels_flat_np):
    import jax
    import jax.numpy as jnp

    cpu = jax.devices("cpu")[0]
    with jax.default_device(cpu):
        labels_flat = jnp.asarray(labels_flat_np)
        key = jax.random.key(42)
        k1, k2 = jax.random.split(key)
        scores = jax.random.uniform(k1, (P,))
        class_mask = (
            labels_flat[None, :]
            == jnp.arange(NUM_CLASSES, dtype=labels_flat.dtype)[:, None]
        )
        masked_scores = jnp.where(class_mask, scores[None, :], -1.0)
        _, idx = jax.lax.top_k(masked_scores, MAX_SAMPLES)
        sampled_idx = idx.reshape(-1)
        row_scores = jax.random.uniform(k2, (N, MAX_SAMPLES))
        _, sel = jax.lax.top_k(row_scores, MAX_VIEWS)
        block_start = (jnp.arange(N) // MAX_SAMPLES) * MAX_SAMPLES
        pos_cols = sel + block_start[:, None]
        return np.asarray(sampled_idx), np.asarray(pos_cols)


def _ring_runs(sb, lo, hi):
    """Contiguous psum-ring runs for chunk cols [lo,hi);
    ring col of chunk col x = (sb*512 + x) mod RS."""
    base = sb * 512
    cuts = [w - base for w in (RS, 2 * RS) if lo < w - base < hi]
    edges = [lo] + cuts + [hi]
    return [(((base + l) % RS), l, h) for l, h in zip(edges[:-1], edges[1:])]


def _build_program():
    """v4: PE = main matmuls only, into an 8-bank psum ring.
    exp split: DVE schraudolph (cols [0,V), int16 bit trick) and ACT exact
    (cols [V,5120)). Columns [0,4096) of every chunk are shipped to DRAM
    (idle DMA engines) and reduced on the host (cs of k1..k3, row sums,
    exact diagonal removal). On-device: only k4 row sums via a 4x-mode
    tensor_scalar accumulate."""
    if _PROGRAM:
        return _PROGRAM

    import concourse.mybir as mybir
    from concourse import bacc, tile

    f32 = mybir.dt.float32
    bf16 = mybir.dt.bfloat16
    i16 = mybir.dt.int16
    AO = mybir.AluOpType

    nc = bacc.Bacc("TRN2", target_bir_lowering=False)

    embR_d = nc.dram_tensor("embR", [128, KC], bf16, kind="ExternalInput")
    fp8 = mybir.dt.float8e4
    i8 = mybir.dt.int8
    adt = fp8 if EA_FP8 else bf16
    e0_d = nc.dram_tensor("e0", [128, 8 * V], i8, kind="ExternalOutput")
    eA_d = nc.dram_tensor("eA", [128, 8 * (4096 - V)], adt, kind="ExternalOutput")
    eB_d = nc.dram_tensor("eB", [128, 8 * 1024], adt, kind="ExternalOutput")

    with tile.TileContext(nc) as tc:
        with (
            tc.tile_pool(name="persist", bufs=1) as pp,
            tc.tile_pool(name="psum", bufs=1, space="PSUM") as ps,
        ):
            embR = pp.tile([128, KC], bf16)
            ering0 = pp.tile([128, 2 * V], i8)              # [0,V) 2 buffers
            eringA = pp.tile([128, 2 * (4096 - V)], adt)    # [V,4096) 2 buffers
            eringB = pp.tile([128, 2 * 1024], adt)          # k4 2 buffers
            ring = ps.tile([128, RS], f32)

            warm = pp.tile([128, 512], bf16)
            for lo, hi in [(0, 1024), (1024, 2048), (2048, 3072),
                           (3072, 4096), (4096, KC)]:
                nc.sync.dma_start(out=embR[:, lo:hi], in_=embR_d[:, lo:hi])
            # PE warm-up: ramp the p-state while input DMAs land; outputs
            # are overwritten by the real matmuls.
            nc.vector.memset(warm[:], 1.0)
            for _ in range(6):
                nc.tensor.matmul(
                    ring[:, 0:512], warm[:, 0:128], warm[:, 0:512],
                    start=True, stop=True,
                )

            for c in range(8):
                sb = (10 * c) % 8
                h0 = (c % 2) * V
                hA = (c % 2) * (4096 - V)
                lhsT = embR[:, c * 128:(c + 1) * 128]

                def mm(x0, x1):
                    for x in range(x0, x1, MM_W):
                        rc = (sb * 512 + x) % RS
                        nc.tensor.matmul(
                            ring[:, rc:rc + MM_W], lhsT, embR[:, x:x + MM_W],
                            start=True, stop=True,
                        )

                # split DVE exp: [0,1024) first so the k4 matmuls (which
                # reuse those two ring slots) only wait on this short read
                mm(0, 1024)
                for rc, l, h in _ring_runs(sb, 0, 1024):
                    nc.vector.tensor_scalar(
                        out=ering0[:, h0 + l:h0 + h],
                        in0=ring[:, rc:rc + (h - l)],
                        scalar1=float(A8_7), scalar2=float(B8C),
                        op0=AO.mult, op1=AO.add,
                    )
                nc.sync.dma_start(
                    out=e0_d[:, c * V:c * V + 1024],
                    in_=ering0[:, h0:h0 + 1024],
                )
                mm(1024, ((V + 511) // 512) * 512)
                for rc, l, h in _ring_runs(sb, 1024, V):
                    nc.vector.tensor_scalar(
                        out=ering0[:, h0 + l:h0 + h],
                        in0=ring[:, rc:rc + (h - l)],
                        scalar1=float(A8_7), scalar2=float(B8C),
                        op0=AO.mult, op1=AO.add,
                    )
                nc.sync.dma_start(
                    out=e0_d[:, c * V + 1024:(c + 1) * V],
                    in_=ering0[:, h0 + 1024:h0 + V],
                )
                mm(((V + 511) // 512) * 512, RS)
                for rc, l, h in _ring_runs(sb, V, 4096):
                    nc.scalar.activation(
                        eringA[:, hA + l - V:hA + h - V],
                        ring[:, rc:rc + (h - l)],
                        mybir.ActivationFunctionType.Exp, scale=float(SCALE),
                    )
                    nc.sync.dma_start(
                        out=eA_d[:, c * (4096 - V) + l - V:
                                 c * (4096 - V) + h - V],
                        in_=eringA[:, hA + l - V:hA + h - V],
                    )
                mm(RS, KC)   # k4; reuses ring slots [0,1024) after DVE read
                hB = (c % 2) * 1024
                for rc, l, h in _ring_runs(sb, 4096, KC):
                    nc.scalar.activation(
                        eringB[:, hB + l - 4096:hB + h - 4096],
                        ring[:, rc:rc + (h - l)],
                        mybir.ActivationFunctionType.Exp, scale=float(SCALE),
                    )
                nc.sync.dma_start(
                    out=eB_d[:, c * 1024:(c + 1) * 1024],
                    in_=eringB[:, hB:hB + 1024],
                )


    nc.finalize()
    _PROGRAM["nc"] = nc
    return _PROGRAM


def _spos_host(emb_n, pos_cols):
    rows = np.repeat(np.arange(N), MAX_VIEWS)
    cols = pos_cols.ravel()
    mask = cols != rows
    rows, cols = rows[mask], cols[mask]
    total = 0.0
    for ofs in range(0, rows.size, 131072):
        r = rows[ofs:ofs + 131072]
        c = cols[ofs:ofs + 131072]
        dots = np.einsum("ij,ij->i", emb_n[r], emb_n[c], dtype=np.float64)
        total += float(np.exp(np.float64(SCALE) * dots).sum())
    return total


def _host_prep(embeddings, labels):
    sampled_idx, pos_cols = _sample_indices_host(labels.reshape(-1))
    hw = H * W
    b = sampled_idx // hw
    h = (sampled_idx % hw) // W
    w = sampled_idx % W
    emb_s = embeddings[b, :, h, w].astype(np.float32)
    norm = np.sqrt(np.sum(emb_s * emb_s, axis=1, dtype=np.float32)).astype(np.float32)
    norm = np.maximum(norm, np.float32(1e-12))
    emb_n = emb_s / norm[:, None]
    embT_bf = np.ascontiguousarray(emb_n.T).astype(ml_dtypes.bfloat16)

    spos = _spos_host(emb_n, pos_cols)

    in_maps = []
    for m in range(N_CORES):
        embR = np.ascontiguousarray(np.roll(embT_bf, -BLK * m, axis=1)[:, :KC])
        in_maps.append({"embR": embR})
    return in_maps, spos


def _combine(results, spos_total):
    rt = []
    csv = []
    for res in results:
        codes = np.asarray(res["e0"]).view(np.int8)
        codes = np.where((codes < 0) | (codes == 127), np.int8(0), codes)
        e0 = codes.view(ml_dtypes.float8_e4m3).astype(np.float32).reshape(128, 8, V)
        eA = np.asarray(res["eA"], dtype=np.float32).reshape(128, 8, 4096 - V)
        eB = np.asarray(res["eB"], dtype=np.float32).reshape(128, 8, 1024)

        rs = (e0.sum(axis=2, dtype=np.float64)
              + eA.sum(axis=2, dtype=np.float64)
              + eB.sum(axis=2, dtype=np.float64))                # [128, 8]
        # subtract the diagonal exactly as shipped (cancels even for
        # saturated codes)
        p_ = np.arange(128)
        for c in range(8):
            rs[p_, c] -= e0[p_, c, c * 128 + p_].astype(np.float64)
        rt.append(rs.T.reshape(-1))                              # u = c*128+p
        cs = np.concatenate([
            e0[:, :, 1024:].sum(axis=(0, 1), dtype=np.float64),
            eA.sum(axis=(0, 1), dtype=np.float64),
        ])
        csv.append(cs.reshape(3, 1024))
    col_sum = np.empty(N, dtype=np.float64)
    for b_ in range(N_CORES):
        s = rt[b_].copy()
        for k in (1, 2, 3):
            s += csv[(b_ - k) % N_CORES][k - 1]
        col_sum[b_ * BLK:(b_ + 1) * BLK] = s
    loss = -np.log(spos_total) + np.mean(np.log(col_sum))
    return np.float32(loss)


def kernel(embeddings: np.ndarray, labels: np.ndarray) -> np.ndarray:
    from concourse.bass_utils import run_bass_kernel_spmd

    prog = _build_program()
    in_maps, spos = _host_prep(np.asarray(embeddings), np.asarray(labels))
    out = run_bass_kernel_spmd(prog["nc"], in_maps, list(range(N_CORES)))
    return _combine(out.results, spos)
